# revision 22
# baseline (speedup 1.0000x reference)
"""Trainium2 Bass kernel for nn_MultiHeadSelfAttentionKV.

Reference computation (B=1, L=4096, D=512, H=8, M=2048, Dh=64):
    q = split_heads(x @ Wq.T); k = split_heads(x @ Wk.T); v = split_heads(x @ Wv.T)
    k_cat = concat([mem_k, k], kv);  v_cat = concat([mem_v, v], kv)
    scores = q @ k_cat.T / sqrt(Dh); masked; attn = softmax(scores)
    y = (attn @ v_cat) merged @ Wo.T;  returns (y, k, v)

Sharding: data-parallel over the 4096 query rows (512 rows per core).
Every core computes the full K/V projections (attention context), attends for
its own 512 query rows over all 8 heads, and emits its row-slice of y, k, v.
No collectives.

Per-core dataflow (all matmuls bf16, accumulation fp32):
  KT   = [mem_k^T | Wk @ xT]          (Dh-major per head: lhsT for scores)
  V    = [mem_v | x @ Wv^T] + ones col (kv-major: stationary for PV)
  qT   = Wq @ xq^T * 1/8
  S^T  = KT_h(slice).T @ qT_h         -> PSUM (kv x q), no transposes needed
  P^T  = exp(S^T)  (ScalarE, PSUM->SBUF bf16)  * mask^T (VectorE/GpSimd)
  O^T  = sum_kv V_aug_h.T @ P^T       -> PSUM accum (65 x 512); row 64 = denom
  normalize: transpose -> per-partition reciprocal -> scale -> transpose back
  y    = aoT.T @ Wo^T

Pair-0 attention is emitted interleaved with the x-projection stream so the
scalar engine (exp — the bottleneck) starts early; each engine's instruction
stream follows emission order.
"""

import numpy as np

B, L, D, H, M, Dh = 1, 4096, 512, 8, 2048, 64
KV = M + L            # 6144
NCORES = 8
Lq = L // NCORES      # 512 query rows per core
NB = KV // 128        # 48 kv blocks
NBM = M // 128        # 16 kv blocks from memory
GRP = 2               # kv blocks per attention batch
NGRP = NB // GRP      # 24
SCALE = 1.0 / 8.0     # 1/sqrt(Dh)


def _build_bass():
    import concourse.bass as bass
    import concourse.mybir as mybir
    import concourse.tile as tile
    from concourse import bacc
    from concourse.bass import ts
    from concourse.masks import make_identity
    from contextlib import ExitStack

    f32 = mybir.dt.float32
    bf16 = mybir.dt.bfloat16
    u8 = mybir.dt.uint8
    Exp = mybir.ActivationFunctionType.Exp
    mult = mybir.AluOpType.mult

    nc = bacc.Bacc()

    # ---- I/O ----
    x_d = nc.dram_tensor("x", [L, D], f32, kind="ExternalInput")
    xq_d = nc.dram_tensor("xq", [Lq, D], f32, kind="ExternalInput")
    wq_d = nc.dram_tensor("wq", [D, D], f32, kind="ExternalInput")
    wk_d = nc.dram_tensor("wk", [D, D], f32, kind="ExternalInput")
    wv_d = nc.dram_tensor("wv", [D, D], f32, kind="ExternalInput")
    wo_d = nc.dram_tensor("wo", [D, D], f32, kind="ExternalInput")
    mk_d = nc.dram_tensor("mem_k", [H, M, Dh], f32, kind="ExternalInput")
    mv_d = nc.dram_tensor("mem_v", [H, M, Dh], f32, kind="ExternalInput")
    mask_d = nc.dram_tensor("mask", [Lq, KV], u8, kind="ExternalInput")
    y_d = nc.dram_tensor("y", [Lq, D], f32, kind="ExternalOutput")
    ko_d = nc.dram_tensor("k_out", [H, Lq, Dh], f32, kind="ExternalOutput")
    vo_d = nc.dram_tensor("v_out", [H, Lq, Dh], f32, kind="ExternalOutput")

    ko_v = ko_d.rearrange("h (t p) c -> t p h c", p=128)  # (4, 128, H, Dh)
    vo_v = vo_d.rearrange("h (t p) c -> t p h c", p=128)

    with tile.TileContext(nc) as tc:
        with ExitStack() as stk:
            const = stk.enter_context(tc.tile_pool(name="const", bufs=1))
            persist = stk.enter_context(tc.tile_pool(name="persist", bufs=1))
            psS = stk.enter_context(tc.tile_pool(name="psS", bufs=3, space="PSUM"))
            psO = stk.enter_context(tc.tile_pool(name="psO", bufs=2, space="PSUM"))
            ptp = stk.enter_context(tc.tile_pool(name="ptp", bufs=3))
            small = stk.enter_context(tc.tile_pool(name="small", bufs=1))
            outp = stk.enter_context(tc.tile_pool(name="outp", bufs=1))
            maskp = stk.enter_context(tc.tile_pool(name="maskp", bufs=1))

            ident = const.tile([128, 128], bf16, tag="ident")
            make_identity(nc, ident)
            identf = const.tile([128, 128], f32, tag="identf")
            make_identity(nc, identf)

            # persistent attention-context tensors
            KT = [persist.tile([128, KV], bf16, tag=f"kt{t}", name=f"kt{t}")
                  for t in range(4)]
            V_all = persist.tile([128, NB, H * 65], bf16, tag="vall")
            qT = [persist.tile([128, Lq], bf16, tag=f"qt{t}", name=f"qt{t}")
                  for t in range(4)]
            aoT = [persist.tile([128, Lq], bf16, tag=f"aot{t}", name=f"aot{t}")
                   for t in range(4)]
            xqT = persist.tile([128, 4, Lq], bf16, tag="xqt")
            WT = {w: persist.tile([128, 4, D], bf16, tag=f"wt{w}", name=f"wt{w}")
                  for w in ("wk", "wv")}
            maskT = maskp.tile([128, NB, Lq], bf16, tag="maskt")

            # ones columns of V_aug (softmax denominator accumulators)
            for h in range(H):
                nc.vector.memset(V_all[:, :, 65 * h + 64], 1.0)

            # --- helpers for staged loading ---
            def load_weight(wd, wt, wpool):
                w_sb = wpool.tile([128, 4, D], bf16, tag="wld", name="w_sb")
                nc.gpsimd.dma_start(out=w_sb[:, :, :],
                                    in_=wd.rearrange("(a p) d -> p a d", p=128))
                for c in range(4):
                    pw = psS.tile([128, 512], bf16, tag="S", name="pw")
                    for t in range(4):
                        nc.tensor.transpose(pw[:, ts(t, 128)],
                                            w_sb[:, t, ts(c, 128)], ident)
                    nc.vector.tensor_copy(out=wt[:, c, :], in_=pw[:, :])

            def load_mask_eighth(ee, mqp):
                ECOLS = KV // 8   # 768 kv cols = 6 blocks
                mq = mqp.tile([128, 4, ECOLS], bf16, tag="mq", name="mq")
                nc.gpsimd.dma_start(
                    out=mq[:, :, :],
                    in_=mask_d[:, ee * ECOLS:(ee + 1) * ECOLS].rearrange(
                        "(a p) c -> p a c", p=128))
                for bl in range(6):
                    pm = psS.tile([128, 512], bf16, tag="S", name="pm")
                    for qt in range(4):
                        nc.tensor.transpose(pm[:, ts(qt, 128)],
                                            mq[:, qt, ts(bl, 128)], ident)
                    nc.vector.tensor_copy(out=maskT[:, 6 * ee + bl, :], in_=pm[:, :])

            def load_memk(h, mkp):
                t, rb = h // 2, (h % 2) * 64
                mk_sb = mkp.tile([128, NBM, Dh], bf16, tag="mkld", name="mk_sb")
                nc.gpsimd.dma_start(
                    out=mk_sb[:, :, :],
                    in_=mk_d[h].rearrange("(b p) c -> p b c", p=128))
                for g in range(4):
                    pm = psS.tile([64, 512], bf16, tag="S", name="pmk")
                    for j in range(4):
                        nc.tensor.transpose(pm[:, ts(j, 128)],
                                            mk_sb[:, 4 * g + j, :], ident)
                    nc.vector.tensor_copy(out=KT[t][rb:rb + 64, ts(g, 512)],
                                          in_=pm[:, :])

            def load_memv(h):
                nc.gpsimd.dma_start(
                    out=V_all[:, 0:NBM, 65 * h:65 * h + 64],
                    in_=mv_d[h].rearrange("(b p) c -> p b c", p=128))

            # ----- attention emission helpers -----
            def emit_attn_group(h, g, pO):
                t, rb = h // 2, (h % 2) * 64
                pS = psS.tile([128, GRP, 512], f32, tag="S", name="pS")
                for j in range(GRP):
                    blk = GRP * g + j
                    nc.tensor.matmul(pS[:, j, :],
                                     KT[t][rb:rb + 64, ts(blk, 128)],
                                     qT[t][rb:rb + 64, :],
                                     start=True, stop=True)
                PT = ptp.tile([128, GRP, 512], bf16, tag="pt", name="PT")
                nc.scalar.activation(PT[:, :, :], pS[:, :, :], Exp)
                eng = nc.gpsimd if (h % 4 == 3) else nc.vector
                eng.tensor_tensor(PT[:, :, :], PT[:, :, :],
                                  maskT[:, GRP * g:GRP * (g + 1), :], mult)
                for j in range(GRP):
                    blk = GRP * g + j
                    nc.tensor.matmul(pO[h][:65, :],
                                     V_all[:, blk, 65 * h:65 * h + 65],
                                     PT[:, j, :],
                                     start=(blk == 0), stop=(blk == NB - 1))

            def emit_normalize(h, pO):
                t, rb = h // 2, (h % 2) * 64
                Ot = small.tile([128, 512], f32, tag="ot", name="Ot")
                nc.vector.tensor_copy(out=Ot[:65, :], in_=pO[h][:65, :])
                pQ = psS.tile([128, GRP, 512], f32, tag="S", name="pQ")
                pQv = pQ[:, 0, :].rearrange("p (a b) -> p a b", a=4)  # (128,4,128)
                for qt in range(4):
                    nc.tensor.transpose(pQv[:, qt, 0:65], Ot[:65, ts(qt, 128)],
                                        identf[0:65, 0:65])
                Oq = small.tile([128, 4, 65], f32, tag="oq", name="Oq")
                nc.vector.tensor_copy(out=Oq[:, :, :], in_=pQv[:, :, 0:65])
                r = small.tile([128, 4, 1], f32, tag="r", name="r")
                nc.vector.reciprocal(out=r[:, :, :], in_=Oq[:, :, 64:65])
                Oqn = small.tile([128, 4, 64], bf16, tag="oqn", name="Oqn")
                for qt in range(4):
                    nc.vector.tensor_scalar_mul(Oqn[:, qt, :], Oq[:, qt, 0:64],
                                                r[:, qt, :])
                pT2 = psS.tile([64, 512], bf16, tag="S", name="pT2")
                for qt in range(4):
                    nc.tensor.transpose(pT2[:, ts(qt, 128)], Oqn[:, qt, :], ident)
                nc.vector.tensor_copy(out=aoT[t][rb:rb + 64, :], in_=pT2[:, :])

            # preload the exp table so the first real exp doesn't pay ~2.7us
            warm = const.tile([1, 8], f32, tag="warm")
            nc.vector.memset(warm, 0.0)
            nc.scalar.activation(warm[:, :], warm[:, :], Exp)

            # --- staged prologue: make pairs 0/1 runnable as early as possible ---
            wldp = stk.enter_context(tc.tile_pool(name="wldp", bufs=1))
            mqp = stk.enter_context(tc.tile_pool(name="mqp", bufs=1))
            mkp = stk.enter_context(tc.tile_pool(name="mkp", bufs=2))

            with tc.tile_pool(name="wqp", bufs=1) as wqp:
                WqT = wqp.tile([128, 4, D], bf16, tag="wtwq", name="WqT")
                load_weight(wq_d, WqT, wldp)
                # xq -> xqT; qT projection
                xqg = wldp.tile([128, 4, D], bf16, tag="wld", name="xqg")
                nc.gpsimd.dma_start(out=xqg[:, :, :],
                                    in_=xq_d.rearrange("(a p) d -> p a d", p=128))
                for c in range(4):
                    pxq = psS.tile([128, 512], bf16, tag="S", name="pxq")
                    for i in range(4):
                        nc.tensor.transpose(pxq[:, ts(i, 128)],
                                            xqg[:, i, ts(c, 128)], ident)
                    nc.vector.tensor_copy(out=xqT[:, c, :], in_=pxq[:, :])
                for t in range(4):
                    ps = psS.tile([128, GRP, 512], f32, tag="S", name="ps_q")
                    for c in range(4):
                        nc.tensor.matmul(ps[:, 0, :], WqT[:, c, ts(t, 128)],
                                         xqT[:, c, :], start=(c == 0), stop=(c == 3))
                    nc.vector.tensor_scalar_mul(qT[t][:, :], ps[:, 0, :], SCALE)

            pOl = {}
            state = {h: 0 for h in range(H)}

            def emit_head_upto(h, g_ready):
                g_ready = min(g_ready, NGRP - 1)
                while state[h] <= g_ready:
                    emit_attn_group(h, state[h], pOl)
                    state[h] += 1

            # pair-0 context only: minimal bytes before the first exp
            load_mask_eighth(0, mqp)
            for h in (0, 1):
                load_memk(h, mkp)
                load_memv(h)
            load_mask_eighth(1, mqp)
            for h in (0, 1):
                pOl[h] = psO.tile([128, 512], f32, tag="O", name=f"pO{h}")
                emit_head_upto(h, 1)
            load_weight(wk_d, WT["wk"], wldp)
            load_weight(wv_d, WT["wv"], wldp)

            # --- x-stream (K^T/V projection) with pair-0 attention mixed in ---
            with tc.tile_pool(name="xsp", bufs=2) as xsp:
                for ltg in range(8):
                    xg = xsp.tile([128, 4, D], bf16, tag="xld", name="xg")
                    nc.gpsimd.dma_start(
                        out=xg[:, :, :],
                        in_=x_d[ltg * 512:(ltg + 1) * 512, :].rearrange(
                            "(a p) d -> p a d", p=128))
                    xtc = xsp.tile([128, 4, 512], bf16, tag="xtc", name="xtc")
                    px = psS.tile([128, 2048], bf16, tag="S", name="px")
                    for c in range(4):
                        for i in range(4):
                            nc.tensor.transpose(px[:, c * 512 + i * 128:
                                                   c * 512 + (i + 1) * 128],
                                                xg[:, i, ts(c, 128)], ident)
                    nc.vector.tensor_copy(
                        out=xtc[:, :, :],
                        in_=px.rearrange("p (a b) -> p a b", a=4))
                    # K^T projection chunk -> KT[t][:, 2048+512*ltg : ...] (ACT copies)
                    for th in range(2):
                        ps = psS.tile([128, GRP, 512], f32, tag="S", name="ps_k")
                        for tt in range(2):
                            t = 2 * th + tt
                            for c in range(4):
                                nc.tensor.matmul(ps[:, tt, :], WT["wk"][:, c, ts(t, 128)],
                                                 xtc[:, c, :], start=(c == 0), stop=(c == 3))
                        for tt in range(2):
                            t = 2 * th + tt
                            nc.scalar.copy(
                                out=KT[t][:, M + 512 * ltg: M + 512 * (ltg + 1)],
                                in_=ps[:, tt, :])
                    # V projection: kv blocks 16+4*ltg .. +4 (DVE copies)
                    for ih in range(2):
                        ps = psS.tile([128, GRP, 512], f32, tag="S", name="ps_v")
                        for ii in range(2):
                            i = 2 * ih + ii
                            for c in range(4):
                                nc.tensor.matmul(ps[:, ii, :], xtc[:, c, ts(i, 128)],
                                                 WT["wv"][:, c, :], start=(c == 0), stop=(c == 3))
                        for ii in range(2):
                            i = 2 * ih + ii
                            nc.vector.tensor_copy(
                                out=V_all[:, NBM + 4 * ltg + i, :].rearrange(
                                    "p (h c) -> p h c", c=65)[:, :, 0:64],
                                in_=ps[:, ii, :].rearrange("p (h c) -> p h c", c=64))
                    # stage context for the post-stream pairs
                    if ltg < 6:
                        load_mask_eighth(2 + ltg, mqp)
                    if ltg < 6 and ltg % 2 == 0:
                        hh = 2 + ltg // 2 * 2
                        for h in (hh, hh + 1):
                            load_memk(h, mkp)
                            load_memv(h)


            for h in (0, 1):
                emit_head_upto(h, NGRP - 1)
            for h in (0, 1):
                emit_normalize(h, pOl)

            # --- pairs 1-3 at full rate (all context resident) ---
            for hp in range(1, 4):
                heads = (2 * hp, 2 * hp + 1)
                pO = {h: psO.tile([128, 512], f32, tag="O", name=f"pO{h}")
                      for h in heads}
                for g in range(NGRP):
                    for h in heads:
                        emit_attn_group(h, g, pO)
                for h in heads:
                    emit_normalize(h, pO)

            # --- load Wo late (only needed for phase C) ---
            wop = stk.enter_context(tc.tile_pool(name="wop", bufs=1))
            WoT = wop.tile([128, 4, D], bf16, tag="wot", name="WoT")
            load_weight(wo_d, WoT, wldp)

            # --- S6: k, v row-major for own rows -> k_out / v_out ---
            for wname, wt, od in (("k", WT["wk"], ko_v), ("v", WT["wv"], vo_v)):
                for qt in range(4):
                    ps = psS.tile([128, GRP, 512], f32, tag="S", name="ps_kv")
                    for c in range(4):
                        nc.tensor.matmul(ps[:, 0, :], xqT[:, c, ts(qt, 128)],
                                         wt[:, c, :], start=(c == 0), stop=(c == 3))
                    kv_sb = outp.tile([128, D], f32, tag="ob", name="kv_sb")
                    nc.scalar.copy(out=kv_sb[:, :], in_=ps[:, 0, :])
                    nc.sync.dma_start(
                        out=od[qt],
                        in_=kv_sb.rearrange("p (h c) -> p h c", c=64))

            # ---------------- Phase C: output projection ----------------
            for qt in range(4):
                ps = psS.tile([128, GRP, 512], f32, tag="S", name="ps_y")
                for t in range(4):
                    nc.tensor.matmul(ps[:, 0, :], aoT[t][:, ts(qt, 128)], WoT[:, t, :],
                                     start=(t == 0), stop=(t == 3))
                y_sb = outp.tile([128, D], f32, tag="ob", name="y_sb")
                nc.scalar.copy(out=y_sb[:, :], in_=ps[:, 0, :])
                nc.sync.dma_start(out=y_d[ts(qt, 128), :], in_=y_sb[:, :])

    nc.compile()
    return nc


_NC_CACHE = None
_LAST_RESULTS = None


def _get_nc():
    global _NC_CACHE
    if _NC_CACHE is None:
        _NC_CACHE = _build_bass()
    return _NC_CACHE


def kernel(x, mem_k, mem_v, attn_mask, Wq, Wk, Wv, Wo):
    from concourse.bass_utils import run_bass_kernel_spmd

    x = np.asarray(x, dtype=np.float32).reshape(L, D)
    mem_k = np.ascontiguousarray(np.asarray(mem_k, dtype=np.float32).reshape(H, M, Dh))
    mem_v = np.ascontiguousarray(np.asarray(mem_v, dtype=np.float32).reshape(H, M, Dh))
    mask = np.asarray(attn_mask).reshape(L, KV).astype(np.uint8)
    Wq = np.ascontiguousarray(np.asarray(Wq, dtype=np.float32))
    Wk = np.ascontiguousarray(np.asarray(Wk, dtype=np.float32))
    Wv = np.ascontiguousarray(np.asarray(Wv, dtype=np.float32))
    Wo = np.ascontiguousarray(np.asarray(Wo, dtype=np.float32))

    in_maps = []
    for i in range(NCORES):
        r0 = i * Lq
        in_maps.append({
            "x": x,
            "xq": np.ascontiguousarray(x[r0:r0 + Lq]),
            "wq": Wq, "wk": Wk, "wv": Wv, "wo": Wo,
            "mem_k": mem_k, "mem_v": mem_v,
            "mask": np.ascontiguousarray(mask[r0:r0 + Lq]),
        })

    nc = _get_nc()
    res = run_bass_kernel_spmd(nc, in_maps, core_ids=list(range(NCORES)))
    global _LAST_RESULTS
    _LAST_RESULTS = res

    y = np.empty((L, D), dtype=np.float32)
    k = np.empty((H, L, Dh), dtype=np.float32)
    v = np.empty((H, L, Dh), dtype=np.float32)
    for i, r in enumerate(res.results):
        r0 = i * Lq
        y[r0:r0 + Lq] = r["y"]
        k[:, r0:r0 + Lq] = r["k_out"]
        v[:, r0:r0 + Lq] = r["v_out"]
    return (y.reshape(B, L, D), k.reshape(B, H, L, Dh), v.reshape(B, H, L, Dh))
